# revision 46
# baseline (speedup 1.0000x reference)
"""Trainium2 Bass kernel for nn_Net_18906446037087 (snntorch Leaky SNN layer).

Reference semantics (per batch element, 255 steps, f32):
    cur = x @ W.T                         # [B, 1]
    m_0 = 0
    m_{t+1} = (0.95*m_t + cur) * (m_t <= 1)
    spk_{t+1} = (m_{t+1} > 1)
Outputs: (spk_rec, mem_rec), each [255, B, 1] f32.

Sharding: pure data parallel over batch across 8 cores (B=65536 -> 8192/core).
Timeline: 260us (v1, PE-transpose matvec + serial DVE scan) -> 177 -> 170 ->
117 -> 104us (this version; the last step emits each block's per-tile ops
one block late so ACT/Pool always hold ready work while DVE runs the next
block's scount/kcalc phase).

Key insight: cur is constant across steps, so each element's trajectory is
PERIODIC: it climbs cur*s_j (s_j = (1-b^j)/(1-b)), first crosses 1 at step
k = k(cur), resets to 0, and repeats with period P = k+1 (P > 255 when
20*cur <= 1, i.e. no reset in the horizon). The reset indicator
rho_t = [t mod P != 0] is therefore known in advance and the recurrence
becomes the data-independent AFFINE scan

    m_t = (BETA*rho_t) * m_{t-1} + (cur*rho_t),

which runs as ONE tensor_tensor_scan instruction per 128 elements (elements
on partitions, time along the free axis). The serial 510-op DVE chain of
v1-v3 (semaphore-latency-bound at ~220ns/op) disappears entirely. The scan
arithmetic is step-for-step identical to the oracle's f32 ops (rho=1 gives
fl(fl(BETA*m)+cur), rho=0 gives exactly 0).

Per-element pipeline (all [128,255] ops, split across DVE/ACT/Pool with
fixed per-tile engine maps so the in-order engine streams pipeline cleanly):
  scount: k-1 = sum_j [cur*s_j <= 1] via one stt+accum_out against a host
          s_j table (exact integer count; saturates to P=257 for
          non-crossing elements; no transcendentals, no act-table loads).
  q = (t+0.5)*(1/P) - 0.5 ; F = round(q) via the fp32 +2^23 magic constant
          (robust floor(t/P): the +-0.5/P midpoint margin swamps fp error);
  dt = q - F ; a = BETA*[dt >= thr] (thr = 1/P - 0.5, midpoint-safe);
  b = a*(cur/BETA) ; m = tensor_tensor_scan(a, b, 0, mult, add).

Matvec: x host-transposed and fp16 (end-to-end rel err 2.27e-3 vs the 2e-2
gate; fp16 runs the PE at 1 cycle/row vs 4 for fp32 and halves x DMA to
~36us). x streams in 8 batch-column blocks, one big host-pre-arranged DMA
per block (2KB descriptors; block 0 split by chunk rows so accumulation
starts early; PE warmed up by dummy matmuls so the p-state ramp is over
before real work). Stationary W [cl,1] fp16, 7 chunk matmuls accumulate in
PSUM [1,512] per 512-batch tile. cur relayout to [128,64] (batch e at
[e%128, e//128]) avoids the busy DMA queue entirely: ACT copies PSUM lines
to SBUF, PE transposes [1,128] slices into PSUM columns, ACT copies them
into cur. Output: one DMA per 4 tiles into DRAM [128, 64*255] tile-major
(1KB descriptors), generation alternating SP HWDGE / Pool SWDGE; host
reassembles [255, B] and derives spk_rec = mem_rec > 1 (comparison on the
full-precision f32 membrane values).
"""
import sys
if "/opt/trn_rl_repo" not in sys.path:
    sys.path.insert(0, "/opt/trn_rl_repo")

import math
import numpy as np
from contextlib import ExitStack

import concourse.bass as bass
import concourse.bacc as bacc
import concourse.mybir as mybir
import concourse.tile as tile
from concourse.bass_utils import run_bass_kernel_spmd

F32 = mybir.dt.float32
F16 = mybir.dt.float16
ALU = mybir.AluOpType
ACTF = mybir.ActivationFunctionType

N_CORES = 8
B_FULL = 65536
B_CORE = B_FULL // N_CORES          # 8192
D = 784
NUM_STEPS = 255
BETA = 0.95
THRESHOLD = 1.0
LNB = math.log(BETA)

BT = 512                             # psum free size
NBLK = 8                             # x batch-column blocks
BLKW = B_CORE // NBLK                # 1024 batch per block
TPB = BLKW // 128                    # 8 tiles (of 128 elements) per block
NTILE = B_CORE // 128                # 64
QUAD = 4                             # tiles per output DMA


def _build():
    nc = bacc.Bacc("TRN2", target_bir_lowering=False, debug=False,
                   num_devices=N_CORES)
    xb_d = nc.dram_tensor("xb", [128, NBLK * 6 * BLKW], F16, kind="ExternalInput")
    xs_d = nc.dram_tensor("xs", [16, B_CORE], F16, kind="ExternalInput")
    w_d = nc.dram_tensor("w", [128, 7], F16, kind="ExternalInput")
    iota_d = nc.dram_tensor("iota", [128, NUM_STEPS], F32, kind="ExternalInput")
    stab_d = nc.dram_tensor("stab", [128, NUM_STEPS], F32, kind="ExternalInput")
    ones_d = nc.dram_tensor("ones", [128, NUM_STEPS], F32, kind="ExternalInput")
    ident_d = nc.dram_tensor("ident", [128, 128], F32, kind="ExternalInput")
    mem_d = nc.dram_tensor("mem", [128, NTILE * NUM_STEPS], F32,
                           kind="ExternalOutput")

    with tile.TileContext(nc) as tc, ExitStack() as ctx:
        const = ctx.enter_context(tc.tile_pool(name="const", bufs=1))
        xpool = ctx.enter_context(tc.tile_pool(name="xpool", bufs=4))
        kpool = ctx.enter_context(tc.tile_pool(name="kpool", bufs=2))
        abpool = ctx.enter_context(tc.tile_pool(name="abpool", bufs=10))
        mpool = ctx.enter_context(tc.tile_pool(name="mpool", bufs=6))
        psum = ctx.enter_context(tc.tile_pool(name="psum", bufs=4, space="PSUM"))
        psum2 = ctx.enter_context(tc.tile_pool(name="psum2", bufs=2, space="PSUM"))
        psumw = ctx.enter_context(tc.tile_pool(name="psumw", bufs=1, space="PSUM"))

        w_t = const.tile([128, 7], F16)
        nc.sync.dma_start(w_t[:], w_d[:])
        T_t = const.tile([128, NUM_STEPS], F32, name="T_t")
        nc.sync.dma_start(T_t[:], iota_d[:])
        S_t = const.tile([128, NUM_STEPS], F32, name="S_t")
        nc.sync.dma_start(S_t[:], stab_d[:])
        ones_t = const.tile([128, NUM_STEPS], F32, name="ones_t")
        nc.sync.dma_start(ones_t[:], ones_d[:])
        id_t = const.tile([128, 128], F32, name="id_t")
        nc.sync.dma_start(id_t[:], ident_d[:])
        warm = const.tile([128, 512], F16, name="warm")
        nc.gpsimd.memset(warm[:], 0.0)
        wps = psumw.tile([1, 512], F32, tag="warmps", name="warmps")
        for i in range(8):
            nc.tensor.matmul(wps[:, :], warm[:, 0:1], warm[:],
                             start=(i == 0), stop=(i == 7))

        # x: per block one big DMA (6x128 feature rows side by side) + small
        xbig, xsml = [], []
        for k in range(NBLK):
            xb = xpool.tile([128, 6 * BLKW], F16, tag="xb", name=f"xb{k}")
            src_sl = xb_d[:, k * 6 * BLKW:(k + 1) * 6 * BLKW]
            if k == 0:
                # split by chunk rows so b-tile 0's accumulation can start
                # as soon as the first three chunks land
                cut = 3 * BLKW
                nc.sync.dma_start(xb[:, :cut], src_sl[:, :cut])
                nc.sync.dma_start(xb[:, cut:], src_sl[:, cut:])
            else:
                nc.sync.dma_start(xb[:], src_sl)
            xs = xpool.tile([16, BLKW], F16, tag="xs", name=f"xs{k}")
            nc.sync.dma_start(xs[:], xs_d[:, k * BLKW:(k + 1) * BLKW])
            xbig.append(xb)
            xsml.append(xs)

        lines = const.tile([1, B_CORE], F32, name="lines")
        cur_t = const.tile([128, NTILE], F32, name="cur")
        invP_t = const.tile([128, NTILE], F32, name="invP_t")
        thr_t = const.tile([128, NTILE], F32, name="thr_t")
        cb_t = const.tile([128, NTILE], F32, name="cb_t")
        load = {"dve": 0.0, "act": 0.0, "pool": 0.0}
        COST = {"dve": 326.0, "act": 395.0, "pool": 450.0}

        def pick(allowed):
            e = min(allowed, key=lambda e: load[e] + COST[e])
            load[e] += COST[e]
            return e

        def matvec_block(k):
            for bi in range(2):
                bt = 2 * k + bi
                bank = psum.tile([1, BT], F32, tag="bank", name=f"bank{bt}")
                for ci in range(7):
                    cl = 128 if ci < 6 else 16
                    off = bi * BT
                    rhs = (xbig[k][:, ci * BLKW + off:ci * BLKW + off + BT]
                           if ci < 6 else xsml[k][:, off:off + BT])
                    nc.tensor.matmul(bank[:, :], w_t[:cl, ci:ci + 1], rhs,
                                     start=(ci == 0), stop=(ci == 6))
                sl = lines[:, bt * BT:(bt + 1) * BT]
                nc.scalar.copy(sl, bank[:, :])
            # relayout via PE transposes (no DMA: the DMA queue is busy
            # streaming x): lines[1,128] slices -> PSUM [128,1] columns,
            # then one ACT copy lands cur_t[p, j] = cur[128*j + p]
            cb = psum2.tile([128, TPB], F32, tag="curbank", name=f"cb{k}")
            for gi in range(TPB):
                nc.tensor.transpose(
                    cb[:, gi:gi + 1],
                    lines[:, k * BLKW + gi * 128:k * BLKW + (gi + 1) * 128],
                    id_t[:1, :1])
            nc.scalar.copy(cur_t[:, k * TPB:(k + 1) * TPB], cb[:, :])
            load["act"] += 1290.0

        epool = ctx.enter_context(tc.tile_pool(name="epool", bufs=2))

        def kcalc_block(k):
            """Crossing count per element: k-1 = sum_j [cur*s_j <= 1] in one
            stt+accum per tile; then P = count+2, invP, thr, cb columns.
            Saturates naturally: cur <= 0.05 gives count 255 -> P 257 > 255."""
            c_sl = cur_t[:, k * TPB:(k + 1) * TPB]
            invP_sl = invP_t[:, k * TPB:(k + 1) * TPB]
            thr_sl = thr_t[:, k * TPB:(k + 1) * TPB]
            cb_sl = cb_t[:, k * TPB:(k + 1) * TPB]
            kkb = kpool.tile([128, TPB], F32, tag="kkb", name=f"kkb{k}")
            for gi in range(TPB):
                e = epool.tile([128, NUM_STEPS], F32, tag="e", name=f"e{k}_{gi}")
                nc.vector.scalar_tensor_tensor(
                    e[:], S_t[:], c_sl[:, gi:gi + 1], ones_t[:],
                    ALU.mult, ALU.is_le, accum_out=kkb[:, gi:gi + 1])
            load["dve"] += TPB * 326.0
            nc.vector.tensor_scalar(kkb[:], kkb[:], 2.0, None, ALU.add)
            nc.vector.reciprocal(invP_sl, kkb[:])
            nc.vector.tensor_scalar(thr_sl, invP_sl, 1.0, -0.5,
                                    ALU.mult, ALU.add)
            nc.vector.tensor_scalar(cb_sl, c_sl, 1.0 / BETA, None, ALU.mult)
            load["dve"] += 500.0

        mq = [None]

        def ts_on(eng, *args):
            (nc.vector if eng == "dve" else nc.gpsimd).tensor_scalar(*args)

        def tile_scan(g):
            """Affine scan for batch elements [128g, 128g+128):
            F = floor(t/P) (exact via +0.5 midpoint + magic round),
            rho = [frac >= thr], m = scan(beta*rho, cur*rho)."""
            q = g % QUAD
            if q == 0:
                mq[0] = mpool.tile([128, QUAD * NUM_STEPS], F32, tag="mq",
                                   name=f"mq{g}")
            w = abpool.tile([128, 5 * NUM_STEPS], F32, tag="ab", name=f"ab{g}")
            qt, ft, dt, a, b = (w[:, i * NUM_STEPS:(i + 1) * NUM_STEPS]
                                for i in range(5))
            invP = invP_t[:, g:g + 1]
            # fixed per-tile op->engine maps (3-cycle): in-order engine
            # streams see a regular pattern, so cross-engine waits pipeline
            # instead of head-of-line blocking
            MAPS = [
                {"q": "act", "F": "pool", "dt": "dve", "a": "pool", "b": "act"},
                {"q": "act", "F": "dve", "dt": "pool", "a": "dve", "b": "act"},
                {"q": "act", "F": "pool", "dt": "dve", "a": "pool", "b": "act"},
            ]
            m = MAPS[g % 3]
            if m["q"] == "act":
                nc.scalar.activation(qt, T_t[:], ACTF.Copy, scale=invP,
                                     bias=-0.5)
            else:
                ts_on(m["q"], qt, T_t[:], invP, -0.5, ALU.mult, ALU.add)
            ts_on(m["F"], ft, qt, 12582912.0, -12582912.0, ALU.add, ALU.add)
            (nc.vector if m["dt"] == "dve" else nc.gpsimd).tensor_tensor(
                dt, qt, ft, ALU.subtract)
            ts_on(m["a"], a, dt, thr_t[:, g:g + 1], BETA, ALU.is_ge, ALU.mult)
            if m["b"] == "act":
                nc.scalar.activation(b, a, ACTF.Copy, scale=cb_t[:, g:g + 1])
            else:
                ts_on(m["b"], b, a, cb_t[:, g:g + 1], None, ALU.mult)
            msl = mq[0][:, q * NUM_STEPS:(q + 1) * NUM_STEPS]
            nc.vector.tensor_tensor_scan(msl, a, b, 0.0, ALU.mult, ALU.add)
            load["dve"] += COST["dve"]
            if q == QUAD - 1:
                g0 = g - (QUAD - 1)
                eng = nc.sync if (g0 // QUAD) % 2 == 0 else nc.gpsimd
                eng.dma_start(
                    mem_d[:, g0 * NUM_STEPS:(g0 + QUAD) * NUM_STEPS], mq[0][:])

        # stagger: half of each block's tiles are emitted after the NEXT
        # block's kcalc, so ACT/Pool always hold ready tile work while DVE
        # runs the (DVE-only) scount/cols phase of the next block
        pend = []
        for k in range(NBLK):
            matvec_block(k)
            kcalc_block(k)
            for g in pend:
                tile_scan(g)
            pend = [k * TPB + gi for gi in range(TPB)]
        for g in pend:
            tile_scan(g)

    nc.compile()
    return nc


_NC_CACHE = None


def _get_nc():
    global _NC_CACHE
    if _NC_CACHE is None:
        _NC_CACHE = _build()
    return _NC_CACHE


def _prep_inputs(x, W):
    x = np.asarray(x, dtype=np.float32)
    W = np.asarray(W, dtype=np.float32).reshape(-1)
    assert x.shape == (B_FULL, D) and W.shape == (D,)
    wpad = np.zeros(896, np.float16)
    wpad[:D] = W.astype(np.float16)
    wcol = np.ascontiguousarray(wpad.reshape(7, 128).T)
    iota = np.tile(np.arange(1, NUM_STEPS + 1, dtype=np.float32) + 0.5, (128, 1))
    j = np.arange(1, NUM_STEPS + 1, dtype=np.float64)
    stab = np.tile(((1.0 - BETA ** j) / (1.0 - BETA)).astype(np.float32), (128, 1))
    ones = np.ones((128, NUM_STEPS), np.float32)
    ident = np.eye(128, dtype=np.float32)
    x16 = x.astype(np.float16)
    in_maps = []
    for d in range(N_CORES):
        xc = x16[d * B_CORE:(d + 1) * B_CORE]
        # xb[p, (k, c, w)] = x[k*BLKW + w, c*128 + p]
        xb = np.ascontiguousarray(
            xc[:, :768].reshape(NBLK, BLKW, 6, 128)
            .transpose(3, 0, 2, 1).reshape(128, NBLK * 6 * BLKW))
        xs = np.ascontiguousarray(xc[:, 768:784].T)
        in_maps.append({"xb": xb, "xs": xs, "w": wcol, "iota": iota,
                        "stab": stab, "ones": ones, "ident": ident})
    return in_maps


def kernel(x, W, _trace=False, _trace_kwargs=None):
    nc = _get_nc()
    in_maps = _prep_inputs(x, W)
    res = run_bass_kernel_spmd(nc, in_maps, list(range(N_CORES)),
                               trace=_trace, **(_trace_kwargs or {}))
    # per-core [128, 64*255] tile-major -> [255, 8192], batch e = 128g + p
    mem = np.concatenate(
        [res.results[d]["mem"].reshape(128, NTILE, NUM_STEPS)
         .transpose(2, 1, 0).reshape(NUM_STEPS, B_CORE)
         for d in range(N_CORES)], axis=1)
    mem_rec = np.ascontiguousarray(mem.reshape(NUM_STEPS, B_FULL, 1))
    spk_rec = (mem_rec > np.float32(THRESHOLD)).astype(np.float32)
    if _trace:
        return (spk_rec, mem_rec), res
    return spk_rec, mem_rec
